# revision 13
# baseline (speedup 1.0000x reference)
"""Trainium2 Bass kernel for GQA attention (nn_Attention_56083682951967).

Sharding: tensor-parallel over KV heads — core c owns kv-head c and q-heads
4c..4c+3 (wq/wk/wv output-dim shard, activations replicated). After a
per-batch AllToAll of attention outputs, core c projects 256 tokens of each
batch against the full wo; the host reassembles token order.

Performance structure (v2):
 * Attention processes head PAIRS: the two heads of an xq tile live in
   partition ranges 0:63 / 64:127, so their 64-contraction score matmuls
   land in disjoint PE row-groups and execute CONCURRENTLY (row tiling).
 * The softmax exp is split across two engines: ScalarE runs native Exp,
   VectorE runs a Schraudolph bit-trick exp (score*A+B cast to int16 ==
   the bf16 bit pattern of exp(score/8)), roughly doubling exp throughput,
   which was the serializer that starved the PE into HAM cold-clock.
 * Softmax reciprocal uses the fast approx custom-DVE op (~5x).
 * QKV(b0) streams chunk-by-chunk through 3 parallel PSUM accumulators so
   the PE never waits a full half's DMA; QKV(b1) / O-proj(b0) remain woven
   into the other batch's attention instruction stream (generator weave).
 * Each AllToAll is split per head-pair; the first half fires at the
   attention phase midpoint so only half the collective remains on the
   critical path at phase end.
"""

import math
import numpy as np
import ml_dtypes

import concourse.bass as bass
import concourse.mybir as mybir
import concourse.tile as tile
from concourse import bacc, bass_utils
from concourse.masks import make_identity

BF16 = mybir.dt.bfloat16
F32 = mybir.dt.float32
I16 = mybir.dt.int16
AF = mybir.ActivationFunctionType

DIM, NH, NKV, HD = 2048, 32, 8, 64
B, S = 2, 2048
T = B * S
NC = 8
CF = 4 * HD          # 256 q-features per core
TPB = S // NC        # 256 output tokens per core per batch
NKC = 16             # 128-token k-chunks per batch
NDC = DIM // 128     # 16 contraction chunks

# Schraudolph exp in bf16-bits space: bits16(exp(x/8)) ~= x*SCH_A + SCH_B
SCH_A = 128.0 * 1.4426950408889634 * 0.125   # 2^7 * log2(e) * (1/sqrt(HD))
SCH_B = 16256.0 - 0.0434687 * 128.0
# how many of the 16 odd-head exps per (ft,qt) group go to the DVE
DVE_EXP_N = 6

_cache = {}


def _build_nc():
    nc = bacc.Bacc(None, num_devices=NC, target_bir_lowering=False, debug=False)

    q_xT = nc.declare_dram_parameter("q_xT", [4, NDC, 128, 1024], BF16, isOutput=False)
    kv_xT = nc.declare_dram_parameter("kv_xT", [4, NDC, 128, 1024], BF16, isOutput=False)
    wq = nc.declare_dram_parameter("wq", [DIM, CF], BF16, isOutput=False)
    wkv = nc.declare_dram_parameter("wkv", [DIM, 2 * HD], BF16, isOutput=False)
    wo = nc.declare_dram_parameter("wo", [DIM, DIM], BF16, isOutput=False)
    cq = nc.declare_dram_parameter("cq", [128, T], BF16, isOutput=False)
    sq = nc.declare_dram_parameter("sq", [128, T], BF16, isOutput=False)
    ck = nc.declare_dram_parameter("ck", [64, T], BF16, isOutput=False)
    sk = nc.declare_dram_parameter("sk", [64, T], BF16, isOutput=False)
    out = nc.declare_dram_parameter("out", [2 * TPB, DIM], F32, isOutput=True)

    # per (batch, head-pair) exchange tensors
    a2a_in = [[nc.dram_tensor(f"a2a_in{b}_{ft}", [NC, 128, TPB], BF16)
               for ft in range(2)] for b in range(B)]
    a2a_out = [[nc.dram_tensor(f"a2a_out{b}_{ft}", [NC, 128, TPB], BF16)
                for ft in range(2)] for b in range(B)]

    with tile.TileContext(nc, num_cores=NC) as tc:
        _emit(nc, tc, q_xT, kv_xT, wq, wkv, wo, cq, sq, ck, sk, out,
              a2a_in, a2a_out)
    nc.finalize()
    return nc


class _env:
    """bag of shared handles for the emit helpers."""


def _load_weight_chunk(E, kc):
    nc = E.nc
    for fb in range(2):
        nc.sync.dma_start(
            E.wq_t[kc][fb][:],
            E.wq[kc * 128:(kc + 1) * 128, fb * 128:(fb + 1) * 128])
    nc.sync.dma_start(E.wkv_t[kc][:], E.wkv[kc * 128:(kc + 1) * 128, :])


def _rope_q(E, rp, ps, cqt, sqt, xq_dst):
    """RoPE for one 128-feature x 1024-token Q block (2 heads)."""
    nc = E.nc
    tmp = rp.tile([128, 1024], F32, tag="tmp", name="tmp")
    for blk in range(4):
        src = (blk // 2) * 2 + (1 - blk % 2)
        nc.vector.tensor_copy(tmp[blk * 32:(blk + 1) * 32, :],
                              ps[src * 32:(src + 1) * 32, :])
    m1 = rp.tile([128, 1024], F32, tag="m1", name="m1")
    nc.vector.tensor_mul(m1[:], ps[:], cqt[:])
    m2 = rp.tile([128, 1024], F32, tag="m2", name="m2")
    nc.vector.tensor_mul(m2[:], tmp[:], sqt[:])
    nc.vector.tensor_add(xq_dst[:], m1[:], m2[:])


def _rope_kv(E, rp, pp, ps_kv, ckt, skt, xk_dst, xvT_dst, xv_dst):
    """RoPE for K (duplicated into both partition halves) + V transpose."""
    nc = E.nc
    tmpk = rp.tile([64, 1024], F32, tag="tmpk", name="tmpk")
    nc.vector.tensor_copy(tmpk[0:32, :], ps_kv[32:64, :])
    nc.vector.tensor_copy(tmpk[32:64, :], ps_kv[0:32, :])
    m1k = rp.tile([64, 1024], F32, tag="m1k", name="m1k")
    nc.vector.tensor_mul(m1k[:], ps_kv[0:64, :], ckt[:])
    m2k = rp.tile([64, 1024], F32, tag="m2k", name="m2k")
    nc.vector.tensor_mul(m2k[:], tmpk[:], skt[:])
    nc.vector.tensor_add(xk_dst[0:64, :], m1k[:], m2k[:])
    nc.vector.tensor_add(xk_dst[64:128, :], m1k[:], m2k[:])
    nc.vector.tensor_copy(xvT_dst[:], ps_kv[64:128, :])
    for c8 in range(8):
        tp = pp.tile([128, 64], BF16, tag="tp", name="tp", bufs=2)
        nc.tensor.transpose(tp[:], xvT_dst[:, c8 * 128:(c8 + 1) * 128],
                            E.ident[:])
        nc.vector.tensor_copy(xv_dst[:, c8, 0:64], tp[:])


def _qkv_stream(E, b, xq_b, xk_b, xvT_b, xv_b):
    """Standalone QKV projection (phase 1): chunk-streamed through three
    parallel PSUM accumulators so the PE chases the activation DMA stream
    instead of waiting for a full half to land."""
    nc = E.nc
    with _multi(
            E.tc.tile_pool(name="sqx", bufs=6),
            E.tc.tile_pool(name="skx", bufs=6),
            E.tc.tile_pool(name="srp", bufs=2),
            E.tc.tile_pool(name="sfq", bufs=2),
            E.tc.tile_pool(name="sps", bufs=1, space="PSUM")) as (
            qxp, kxp, rp, fp, pp):
        for half in range(2):
            gbase = b * S + half * 1024
            cqt = fp.tile([128, 1024], BF16, tag="cqt", name="cqt")
            nc.sync.dma_start(cqt[:], E.cq[:, gbase:gbase + 1024])
            sqt = fp.tile([128, 1024], BF16, tag="sqt", name="sqt")
            nc.sync.dma_start(sqt[:], E.sq[:, gbase:gbase + 1024])
            ckt = fp.tile([64, 1024], BF16, tag="ckt", name="ckt")
            nc.sync.dma_start(ckt[:], E.ck[:, gbase:gbase + 1024])
            skt = fp.tile([64, 1024], BF16, tag="skt", name="skt")
            nc.sync.dma_start(skt[:], E.sk[:, gbase:gbase + 1024])

            ps_q = [pp.tile([128, 1024], F32, tag=f"psq{fb}", name=f"psq{fb}")
                    for fb in range(2)]
            for kc in range(NDC):
                qx = qxp.tile([128, 1024], BF16, tag="qx", name="qx")
                nc.sync.dma_start(qx[:], E.q_xT[b * 2 + half, kc, :, :])
                if not E.weights_loaded:
                    _load_weight_chunk(E, kc)
                st, sp_ = kc == 0, kc == NDC - 1
                for fb in range(2):
                    for qq in range(2):
                        nc.tensor.matmul(
                            ps_q[fb][:, qq * 512:(qq + 1) * 512],
                            E.wq_t[kc][fb][:],
                            qx[:, qq * 512:(qq + 1) * 512],
                            start=st, stop=sp_)
            E.weights_loaded = True

            # K/V sub-phase; fb0's rope drain overlaps these matmuls
            _rope_q(E, rp, ps_q[0], cqt, sqt, xq_b[half])
            ps_kv = pp.tile([128, 1024], F32, tag="pskv", name="ps_kv")
            for kc in range(NDC):
                kx = kxp.tile([128, 1024], BF16, tag="kx", name="kx")
                nc.sync.dma_start(kx[:], E.kv_xT[b * 2 + half, kc, :, :])
                st, sp_ = kc == 0, kc == NDC - 1
                for qq in range(2):
                    nc.tensor.matmul(ps_kv[:, qq * 512:(qq + 1) * 512],
                                     E.wkv_t[kc][:],
                                     kx[:, qq * 512:(qq + 1) * 512],
                                     start=st, stop=sp_)
            _rope_q(E, rp, ps_q[1], cqt, sqt, xq_b[2 + half])
            _rope_kv(E, rp, pp, ps_kv, ckt, skt, xk_b[half], xvT_b[half],
                     xv_b[half])


def _qkv_gen(E, b, xq_b, xk_b, xvT_b, xv_b):
    """QKV projection as a generator: yields between small instruction
    groups so the caller can weave it into an attention phase."""
    nc = E.nc
    with _multi(
            E.tc.tile_pool(name=f"p1q{b}", bufs=2),
            E.tc.tile_pool(name=f"p1k{b}", bufs=4),
            E.tc.tile_pool(name=f"rope{b}", bufs=1),
            E.tc.tile_pool(name=f"freqs{b}", bufs=1)) as (qpool, kpool, rp, fp):
        for half in range(2):
            gbase = b * S + half * 1024
            qxc = [qpool.tile([128, 1024], BF16, tag=f"qxc{kc}",
                              name=f"qxc{kc}") for kc in range(NDC)]
            for kc in range(NDC):
                nc.sync.dma_start(qxc[kc][:],
                                  E.q_xT[b * 2 + half, kc, :, :])
                if not E.weights_loaded:
                    _load_weight_chunk(E, kc)
                if kc % 4 == 3:
                    yield
            E.weights_loaded = True
            cqt = fp.tile([128, 1024], BF16, tag="cqt")
            nc.sync.dma_start(cqt[:], E.cq[:, gbase:gbase + 1024])
            sqt = fp.tile([128, 1024], BF16, tag="sqt")
            nc.sync.dma_start(sqt[:], E.sq[:, gbase:gbase + 1024])
            ckt = fp.tile([64, 1024], BF16, tag="ckt")
            nc.sync.dma_start(ckt[:], E.ck[:, gbase:gbase + 1024])
            skt = fp.tile([64, 1024], BF16, tag="skt")
            nc.sync.dma_start(skt[:], E.sk[:, gbase:gbase + 1024])
            yield

            for fb in range(2):
                ps = E.qps.tile([128, 1024], F32, tag="ps", name="ps")
                for kc in range(NDC):
                    st, sp_ = kc == 0, kc == NDC - 1
                    for qq in range(2):
                        nc.tensor.matmul(
                            ps[:, qq * 512:(qq + 1) * 512],
                            E.wq_t[kc][fb][:],
                            qxc[kc][:, qq * 512:(qq + 1) * 512],
                            start=st, stop=sp_)
                    if kc % 2 == 1:
                        yield
                _rope_q(E, rp, ps, cqt, sqt, xq_b[2 * fb + half])
                yield

            ps_kv = E.qps.tile([128, 1024], F32, tag="ps", name="ps_kv")
            for kc in range(NDC):
                kx = kpool.tile([128, 1024], BF16, tag="kx")
                nc.sync.dma_start(kx[:], E.kv_xT[b * 2 + half, kc, :, :])
                st, sp_ = kc == 0, kc == NDC - 1
                for qq in range(2):
                    nc.tensor.matmul(ps_kv[:, qq * 512:(qq + 1) * 512],
                                     E.wkv_t[kc][:],
                                     kx[:, qq * 512:(qq + 1) * 512],
                                     start=st, stop=sp_)
                if kc % 2 == 1:
                    yield

            tmpk = rp.tile([64, 1024], F32, tag="tmpk")
            nc.vector.tensor_copy(tmpk[0:32, :], ps_kv[32:64, :])
            nc.vector.tensor_copy(tmpk[32:64, :], ps_kv[0:32, :])
            m1k = rp.tile([64, 1024], F32, tag="m1k")
            nc.vector.tensor_mul(m1k[:], ps_kv[0:64, :], ckt[:])
            m2k = rp.tile([64, 1024], F32, tag="m2k")
            nc.vector.tensor_mul(m2k[:], tmpk[:], skt[:])
            nc.vector.tensor_add(xk_b[half][0:64, :], m1k[:], m2k[:])
            nc.vector.tensor_add(xk_b[half][64:128, :], m1k[:], m2k[:])
            nc.vector.tensor_copy(xvT_b[half][:], ps_kv[64:128, :])
            yield
            for c8 in range(8):
                tp = E.qps.tile([128, 64], BF16, tag="ps", name="tp")
                nc.tensor.transpose(tp[:],
                                    xvT_b[half][:, c8 * 128:(c8 + 1) * 128],
                                    E.ident[:])
                nc.vector.tensor_copy(xv_b[half][:, c8, 0:64], tp[:])
                if c8 % 4 == 3:
                    yield


def _oproj_gen(E, b, a2a_out_pair, out):
    """output projection for batch b as a generator (woven into the other
    batch's attention). Consumes the two per-head-pair a2a tensors."""
    nc = E.nc
    with _multi(E.tc.tile_pool(name=f"olhs{b}", bufs=2),
                E.tc.tile_pool(name=f"osb{b}", bufs=4)) as (ol_, ob_):
        for mt in range(2):
            lb = ol_.tile([128, NDC * 128], BF16, tag="lb")
            fc_order = [2 * i for i in range(NDC // 2)] + \
                       [2 * i + 1 for i in range(NDC // 2)]
            for n_, fc in enumerate(fc_order):
                nc.sync.dma_start(
                    lb[:, fc * 128:(fc + 1) * 128],
                    a2a_out_pair[fc % 2][fc // 2, :, mt * 128:(mt + 1) * 128])
                if n_ % 4 == 3:
                    yield
            for nt in range(4):
                po = E.ops.tile([128, 512], F32, tag="po", name="po")
                for n_, fc in enumerate(fc_order):
                    nc.tensor.matmul(
                        po[:],
                        lb[:, fc * 128:(fc + 1) * 128],
                        E.wo_sb[:, fc * DIM + nt * 512: fc * DIM + (nt + 1) * 512],
                        start=(n_ == 0), stop=(n_ == NDC - 1))
                    if n_ % 8 == 7:
                        yield
                ob = ob_.tile([128, 512], F32, tag="ob")
                nc.vector.tensor_copy(ob[:], po[:])
                nc.sync.dma_start(
                    out[b * TPB + mt * 128: b * TPB + (mt + 1) * 128,
                        nt * 512:(nt + 1) * 512], ob[:])
                yield


def _attn_phase(E, b, xq_b, xk_b, xv_b, a2a_pair, filler, fill_stride=1,
                mid_cb=None):
    """attention for batch b, head-paired. The two heads of each xq tile
    (partitions 0:63 / 64:127) get concurrently-executing score matmuls
    (disjoint PE row groups) and their exps split across ScalarE / VectorE.
    Calls next(filler) between steps to weave dependency-free matmuls from
    another phase into the PE stream; mid_cb fires after the first head
    pair (for the early half-AllToAll)."""
    nc = E.tc.nc
    cnt = [0]

    def fill(n=1):
        cnt[0] += 1
        if cnt[0] % fill_stride != 0:
            return
        if filler is not None:
            for _ in range(n):
                next(filler, None)

    pending = []

    def flush_pending(np_):
        while pending:
            for side, ld, nm, a2a_w, qt_w in pending.pop(0):
                rb = np_.tile([64, 512], F32, tag=f"rb{side}", name="rb")
                nc.scalar.activation(rb[:], ld[:], AF.Exp, scale=-1.0)
                ab = np_.tile([64, 512], BF16, tag=f"ab{side}", name="ab")
                nc.gpsimd.tensor_mul(ab[:], nm[:], rb[:])
                ro = 0 if side == "e" else 64
                for qq2 in range(2):
                    d = qt_w * 2 + qq2
                    nc.sync.dma_start(
                        a2a_w[d, ro:ro + 64, :],
                        ab[:, qq2 * 256:(qq2 + 1) * 256])

    with _multi(
            E.tc.tile_pool(name=f"scp{b}", bufs=1, space="PSUM"),
            E.tc.tile_pool(name=f"exp{b}", bufs=3),
            E.tc.tile_pool(name=f"norm{b}", bufs=2)) as (sp, ep, np_):
        for ft in range(2):
            a2a_t = a2a_pair[ft]
            for qt in range(4):
                flush_pending(np_)
                xq_t = xq_b[2 * ft + qt // 2]
                qof = (qt % 2) * 512
                acc_e = E.accp.tile([128, 512], F32, tag="acce", name="acc_e")
                acc_o = E.accp.tile([128, 512], F32, tag="acco", name="acc_o")
                exq = {}

                def pv(kp):
                    exe, exo = exq.pop(kp)
                    for j in range(2):
                        kc = 2 * kp + j
                        xvt = xv_b[kc // 8][:, kc % 8, :]
                        st, sp_ = kc == 0, kc == NKC - 1
                        jj = slice(j * 512, (j + 1) * 512)
                        nc.tensor.matmul(acc_e[:], xvt, exe[:, jj],
                                         start=st, stop=sp_)
                        nc.tensor.matmul(acc_o[:], xvt, exo[:, jj],
                                         start=st, stop=sp_)

                for kp in range(NKC // 2):
                    # PV first: gives the exp of pair kp-1 a full extra
                    # sub-step before its sc banks are reused.
                    if kp >= 2:
                        pv(kp - 2)
                    sc_e = sp.tile([128, 1024], F32, tag="sce", name="sc_e")
                    sc_o = sp.tile([128, 1024], F32, tag="sco", name="sc_o")
                    for j in range(2):
                        kc = 2 * kp + j
                        half, klo = kc // 8, (kc % 8) * 128
                        jj = slice(j * 512, (j + 1) * 512)
                        # adjacent 64-contraction matmuls, disjoint row groups
                        nc.tensor.matmul(sc_e[:, jj],
                                         xk_b[half][0:64, klo:klo + 128],
                                         xq_t[0:64, qof:qof + 512],
                                         start=True, stop=True)
                        nc.tensor.matmul(sc_o[:, jj],
                                         xk_b[half][64:128, klo:klo + 128],
                                         xq_t[64:128, qof:qof + 512],
                                         start=True, stop=True)
                    ex_e = ep.tile([128, 1024], BF16, tag="exe", name="ex_e")
                    ex_o = ep.tile([128, 1024], BF16, tag="exo", name="ex_o")
                    nc.scalar.activation(ex_e[:], sc_e[:], AF.Exp, scale=0.125)
                    if kp < DVE_EXP_N:
                        nc.vector.tensor_scalar(
                            ex_o[:].bitcast(I16), sc_o[:], SCH_A, SCH_B,
                            mybir.AluOpType.mult, mybir.AluOpType.add)
                    else:
                        nc.scalar.activation(ex_o[:], sc_o[:], AF.Exp,
                                             scale=0.125)
                    exq[kp] = (ex_e, ex_o)
                    fill()
                pv(NKC // 2 - 2)
                pv(NKC // 2 - 1)

                # evacuate acc fast (ScalarE: Ln of denom, VectorE: bf16
                # numer copy) so the next group's PV can reuse the PSUM
                # banks. 1/denom = Exp(-Ln(denom)); the Exp half is
                # deferred into the next group's exp stream so the ACT
                # table swaps Exp->Ln->Exp once per group, not twice.
                pend = []
                for side, acc in (("e", acc_e), ("o", acc_o)):
                    ld = np_.tile([64, 512], F32, tag=f"ld{side}", name="ld")
                    nc.scalar.activation(ld[:], acc[64:128, :], AF.Ln)
                    nm = np_.tile([64, 512], BF16, tag=f"nm{side}", name="nm")
                    nc.vector.tensor_copy(nm[:], acc[0:64, :])
                    pend.append((side, ld, nm, a2a_t, qt))
                pending.append(pend)
                fill()
            if ft == 0:
                flush_pending(np_)
                if mid_cb is not None:
                    mid_cb()
        flush_pending(np_)
        # drain any remaining filler work (inside the pool scope so the
        # filler's pools close before this phase's — LIFO requirement)
        if filler is not None:
            for _ in filler:
                pass


def _a2a(nc, src, dst):
    nc.gpsimd.collective_compute(
        "AllToAll", mybir.AluOpType.bypass,
        replica_groups=[list(range(NC))],
        ins=[src[:, :, :].opt()],
        outs=[dst[:, :, :].opt()])


def _emit(nc, tc, q_xT, kv_xT, wq, wkv, wo, cq, sq, ck, sk, out,
          a2a_in, a2a_out):
    from contextlib import ExitStack
    es = ExitStack()
    const = es.enter_context(tc.tile_pool(name="const", bufs=1))

    E = _env()
    E.nc, E.tc = nc, tc
    E.q_xT, E.kv_xT, E.cq, E.sq, E.ck, E.sk = q_xT, kv_xT, cq, sq, ck, sk

    # per-chunk weight tiles so the first matmuls depend only on their own
    # DMA; loaded interleaved with the first activation chunks
    E.wq_t = [[const.tile([128, 128], BF16, tag=f"wq{kc}_{fb}",
                          name=f"wq{kc}_{fb}") for fb in range(2)]
              for kc in range(NDC)]
    E.wkv_t = [const.tile([128, 128], BF16, tag=f"wkv{kc}", name=f"wkv{kc}")
               for kc in range(NDC)]
    E.wq, E.wkv = wq, wkv
    E.weights_loaded = False

    xq_b, xk_b, xvT_b, xv_b = [], [], [], []
    for b in range(B):
        xq_b.append([const.tile([128, 1024], BF16, tag=f"xq{b}_{i}",
                                name=f"xq{b}_{i}") for i in range(4)])
        xk_b.append([const.tile([128, 1024], BF16, tag=f"xk{b}_{i}",
                                name=f"xk{b}_{i}") for i in range(2)])
        xvT_b.append([const.tile([64, 1024], BF16, tag=f"xvT{b}_{i}",
                                 name=f"xvT{b}_{i}") for i in range(2)])
        vs = [const.tile([128, 8, 128], BF16, tag=f"xv{b}_{i}",
                         name=f"xv{b}_{i}") for i in range(2)]
        for v in vs:
            nc.vector.memset(v[:, :, 64:128], 1.0)
        xv_b.append(vs)
    E.ident = const.tile([64, 64], BF16, tag="ident")
    make_identity(nc, E.ident[:])

    # phase 1: streamed QKV(b0); xq tile order [fb0h0, fb0h1, fb1h0, fb1h1]
    # (uses all 8 PSUM banks itself, so the attention acc pool opens after)
    _qkv_stream(E, 0, xq_b[0], xk_b[0], xvT_b[0], xv_b[0])

    # PSUM budget (8 banks): attention sc tiles (4 x 512) + acc (2) +
    # woven-phase accumulator qps/ops (2).
    E.accp = es.enter_context(tc.tile_pool(name="accp", bufs=1, space="PSUM"))

    with tc.tile_pool(name="qps", bufs=1, space="PSUM") as qps_pool:
        E.qps = qps_pool
        # attention(b0) with QKV(b1) woven in
        g1 = _qkv_gen(E, 1, xq_b[1], xk_b[1], xvT_b[1], xv_b[1])
        _attn_phase(E, 0, xq_b[0], xk_b[0], xv_b[0],
                    (a2a_in[0][0], a2a_in[0][1]), g1, fill_stride=2,
                    mid_cb=lambda: _a2a(nc, a2a_in[0][0], a2a_out[0][0]))
    _a2a(nc, a2a_in[0][1], a2a_out[0][1])

    # wo residency loaded during attention(b1); O-proj(b0) woven in.
    wop = es.enter_context(tc.tile_pool(name="wop", bufs=1))
    E.wo_sb = wop.tile([128, NDC * DIM], BF16, tag="wo_sb")
    for fc in range(NDC):
        nc.sync.dma_start(E.wo_sb[:, fc * DIM:(fc + 1) * DIM],
                          wo[fc * 128:(fc + 1) * 128, :])
    E.ops = es.enter_context(tc.tile_pool(name="ops", bufs=2, space="PSUM"))

    g2 = _oproj_gen(E, 0, (a2a_out[0][0], a2a_out[0][1]), out)
    _attn_phase(E, 1, xq_b[1], xk_b[1], xv_b[1],
                (a2a_in[1][0], a2a_in[1][1]), g2, fill_stride=4,
                mid_cb=lambda: _a2a(nc, a2a_in[1][0], a2a_out[1][0]))
    _a2a(nc, a2a_in[1][1], a2a_out[1][1])

    for _ in _oproj_gen(E, 1, (a2a_out[1][0], a2a_out[1][1]), out):
        pass
    es.close()


class _multi:
    def __init__(self, *cms):
        self.cms = cms

    def __enter__(self):
        self.vals = [cm.__enter__() for cm in self.cms]
        return self.vals

    def __exit__(self, *a):
        for cm in reversed(self.cms):
            cm.__exit__(*a)
        return False


def _rope_perm(n_heads):
    idx = []
    for h in range(n_heads):
        base = h * HD
        idx.extend([base + 2 * j for j in range(32)])
        idx.extend([base + 2 * j + 1 for j in range(32)])
    return np.array(idx)


def _prep_in_maps(q_x, kv_x, q_freqs_cis, k_freqs_cis, wq, wk, wv, wo):
    bf = ml_dtypes.bfloat16
    def _tile_xT(x):
        t = x.reshape(T, DIM).T.reshape(NDC, 128, 4, 1024)
        return np.ascontiguousarray(t.transpose(2, 0, 1, 3)).astype(bf)

    q_xT = _tile_xT(q_x)
    kv_xT = _tile_xT(kv_x)

    qf = q_freqs_cis.reshape(T, HD)
    kf = k_freqs_cis.reshape(T, HD)
    fcq, fsq = qf[:, :32].T, qf[:, 32:].T
    fck, fsk = kf[:, :32].T, kf[:, 32:].T
    cq = np.ascontiguousarray(np.tile(fcq, (4, 1))).astype(bf)
    sq = np.ascontiguousarray(np.tile(np.vstack([-fsq, fsq]), (2, 1))).astype(bf)
    ck = np.ascontiguousarray(np.tile(fck, (2, 1))).astype(bf)
    sk = np.ascontiguousarray(np.vstack([-fsk, fsk])).astype(bf)

    wq_p = wq[:, _rope_perm(NH)]
    wk_p = wk[:, _rope_perm(NKV)]
    wo_bf = np.ascontiguousarray(wo).astype(bf)

    in_maps = []
    for c in range(NC):
        wq_c = np.ascontiguousarray(wq_p[:, c * CF:(c + 1) * CF]).astype(bf)
        wkv_c = np.ascontiguousarray(
            np.hstack([wk_p[:, c * HD:(c + 1) * HD],
                       wv[:, c * HD:(c + 1) * HD]])).astype(bf)
        in_maps.append({
            "q_xT": q_xT, "kv_xT": kv_xT,
            "wq": wq_c, "wkv": wkv_c, "wo": wo_bf,
            "cq": cq, "sq": sq, "ck": ck, "sk": sk,
        })
    return in_maps


last_results = None


def kernel(q_x, kv_x, q_freqs_cis, k_freqs_cis, mask, wq, wk, wv, wo):
    global last_results
    if "nc" not in _cache:
        _cache["nc"] = _build_nc()
    nc = _cache["nc"]
    in_maps = _prep_in_maps(np.asarray(q_x, np.float32),
                            np.asarray(kv_x, np.float32),
                            np.asarray(q_freqs_cis, np.float32),
                            np.asarray(k_freqs_cis, np.float32),
                            np.asarray(wq, np.float32),
                            np.asarray(wk, np.float32),
                            np.asarray(wv, np.float32),
                            np.asarray(wo, np.float32))
    res = bass_utils.run_bass_kernel_spmd(nc, in_maps, core_ids=list(range(NC)))
    last_results = res
    out_full = np.zeros((T, DIM), np.float32)
    for c in range(NC):
        r = np.asarray(res.results[c]["out"], np.float32)
        for b in range(B):
            out_full[b * S + TPB * c: b * S + TPB * (c + 1)] = \
                r[b * TPB:(b + 1) * TPB]
    return out_full.reshape(B, S, DIM)
